# revision 1
# baseline (speedup 1.0000x reference)
"""Trainium2 Bass kernel for sparse (top-k) attention, nn_Attention_61014305407316.

Full-module kernel: qkv projection -> per-head scaled dots -> per-row top-716
masked softmax -> attn @ V -> output projection (+bias), distributed over 8
NeuronCores.

Sharding: core = (batch b, token-half s). Each core computes the output rows
for its 512 query tokens of its batch (all 8 heads), reading all 1024 k/v
tokens of that batch. Gather is pure concatenation.

Numerics: matmul inputs in fp16 (fp32 PSUM accumulation), dots/softmax-domain
fp32->fp16.  Top-k threshold per row found in exp-domain (E = exp(scale*dots),
monotone) by bracketed counting probes + density-interpolation refines
(N_REFINE tunable).  Masked weights W = E*(E>=u), denominator via fused accum,
normalization folded as a per-partition scale before W^T transpose.
"""
import numpy as np

import concourse.bacc as bacc
import concourse.bass as bass
import concourse.mybir as mybir
import concourse.tile as tile
from concourse.bass_utils import run_bass_kernel_spmd

F32 = mybir.dt.float32
F16 = mybir.dt.float16
AL = mybir.AluOpType
AF = mybir.ActivationFunctionType

# problem constants (hardcoded; kernel.py must be self-contained)
B, NT, DIM = 4, 1024, 512      # batch, tokens, model dim
H, D = 8, 64                   # heads, head dim
NQ = 512                       # query tokens per core (token-half)
KK = 716                       # int(1024 * 0.7) kept per row
SCALE = 0.125                  # D ** -0.5
T0, W0 = -0.5244, 0.15         # probe anchors in scaled-dot domain
ULO = float(np.exp(T0 - W0))
UHI = float(np.exp(T0 + W0))
LAM0 = 1.0 / (2 * W0)          # slope scale for bracket
LAM_MIN, LAM_MAX = 71.3, 1782.0
N_REFINE = 2                   # full-row refine probes after the bracket

N_CORES = 8


def build_nc(n_refine=N_REFINE):
    nc = bacc.Bacc("TRN2", target_bir_lowering=False)

    x_full = nc.dram_tensor("x_full", [NT, DIM], F32, kind="ExternalInput")
    x_half = nc.dram_tensor("x_half", [NQ, DIM], F32, kind="ExternalInput")
    wq_d = nc.dram_tensor("wq", [DIM, DIM], F32, kind="ExternalInput")
    wk_d = nc.dram_tensor("wk", [DIM, DIM], F32, kind="ExternalInput")
    wv_d = nc.dram_tensor("wv", [DIM, DIM], F32, kind="ExternalInput")
    wo_d = nc.dram_tensor("wo", [DIM, DIM], F32, kind="ExternalInput")
    bo_d = nc.dram_tensor("bo", [128, DIM], F32, kind="ExternalInput")
    y_d = nc.dram_tensor("y", [NQ, DIM], F32, kind="ExternalOutput")

    ident_d = nc.inline_tensor(np.eye(128, dtype=np.float16), name="ident16")

    with tile.TileContext(nc) as tc:
        with tc.tile_pool(name="const", bufs=1) as cp:
            # ---------- phase A: loads, casts, transposes ----------
            ident = cp.tile([128, 128], F16)
            nc.sync.dma_start(ident[:], ident_d[:])

            x32 = cp.tile([128, 8, DIM], F32)        # x_full, token-partition
            nc.sync.dma_start(x32[:], x_full.rearrange("(t p) d -> p t d", p=128))
            x16 = cp.tile([128, 8, DIM], F16)
            nc.vector.tensor_copy(x16[:], x32[:])

            xh32 = cp.tile([128, 4, DIM], F32)
            nc.sync.dma_start(xh32[:], x_half.rearrange("(t p) d -> p t d", p=128))
            xh16 = cp.tile([128, 4, DIM], F16)
            nc.vector.tensor_copy(xh16[:], xh32[:])

            # x^T (dim-partition) via PE transpose, [128,128] blocks
            xT = cp.tile([128, 4, NT], F16)          # all 1024 tokens
            xqT = cp.tile([128, 4, NQ], F16)         # local 512 q tokens
            with tc.tile_pool(name="xtp", bufs=2,
                              space=bass.MemorySpace.PSUM) as xtp:
                for c in range(4):
                    for tg in range(2):
                        tps = xtp.tile([128, NQ], F16, tag="xt")
                        for i in range(4):
                            t = 4 * tg + i
                            nc.tensor.transpose(
                                tps[:, 128 * i:128 * (i + 1)],
                                x16[:, t, 128 * c:128 * (c + 1)], ident[:])
                        nc.vector.tensor_copy(xT[:, c, 512 * tg:512 * (tg + 1)], tps[:])
                for c in range(4):
                    tps = xtp.tile([128, NQ], F16, tag="xt")
                    for i in range(4):
                        nc.tensor.transpose(
                            tps[:, 128 * i:128 * (i + 1)],
                            xh16[:, i, 128 * c:128 * (c + 1)], ident[:])
                    nc.vector.tensor_copy(xqT[:, c, :], tps[:])

            # weights -> fp16, dim-chunk partition layout [128, 4, 512]
            w16 = {}
            for name, dram in (("wq", wq_d), ("wk", wk_d), ("wv", wv_d), ("wo", wo_d)):
                w32 = cp.tile([128, 4, DIM], F32, tag="wstage")
                nc.sync.dma_start(w32[:], dram.rearrange("(c p) m -> p c m", p=128))
                w16[name] = cp.tile([128, 4, DIM], F16, tag=f"w16_{name}",
                                    name=f"w16_{name}")
                nc.vector.tensor_copy(w16[name][:], w32[:])

            bbc = cp.tile([128, DIM], F32)
            nc.sync.dma_start(bbc[:], bo_d[:])

            # per-partition fp32 bias constants for the ACT sign probes
            blo = cp.tile([128, 1], F32)
            nc.vector.memset(blo[:], -ULO)
            bhi = cp.tile([128, 1], F32)
            nc.vector.memset(bhi[:], -UHI)

            # ---------- phase B: qkv projections (fp16 matmuls) ----------
            qT = cp.tile([128, 4, NQ], F16)    # q^T: [q-col-group, g, local tok]
            kT = cp.tile([128, 4, NT], F16)    # k^T: [k-col-group, g, tok]
            v16 = cp.tile([128, 8, DIM], F16)  # v:   [tok-in-tile, tok-tile, v-col]
            hoT = cp.tile([128, 4, NQ], F16)   # head-out^T accumulated per head
            ysb = cp.tile([128, 4, DIM], F32)

            with tc.tile_pool(name="bpsum", bufs=4, space=bass.MemorySpace.PSUM) as bp:
                for g in range(4):
                    ps = bp.tile([128, NQ], F32, tag="bp")
                    for c in range(4):
                        nc.tensor.matmul(ps[:], w16["wq"][:, c, 128 * g:128 * (g + 1)],
                                         xqT[:, c, :], start=(c == 0), stop=(c == 3))
                    nc.scalar.copy(qT[:, g, :], ps[:])
                for g in range(4):
                    for jh in range(2):
                        ps = bp.tile([128, NQ], F32, tag="bp")
                        for c in range(4):
                            nc.tensor.matmul(ps[:], w16["wk"][:, c, 128 * g:128 * (g + 1)],
                                             xT[:, c, 512 * jh:512 * (jh + 1)],
                                             start=(c == 0), stop=(c == 3))
                        nc.scalar.copy(kT[:, g, 512 * jh:512 * (jh + 1)], ps[:])
                for t in range(8):
                    ps = bp.tile([128, NQ], F32, tag="bp")
                    for c in range(4):
                        nc.tensor.matmul(ps[:], xT[:, c, 128 * t:128 * (t + 1)],
                                         w16["wv"][:, c, :], start=(c == 0), stop=(c == 3))
                    nc.scalar.copy(v16[:, t, :], ps[:])

            # ---------- phase C: per-head attention ----------
            with (
                tc.tile_pool(name="dpsum", bufs=2, space=bass.MemorySpace.PSUM) as dp,
                tc.tile_pool(name="tpsum", bufs=2, space=bass.MemorySpace.PSUM) as tp,
                tc.tile_pool(name="avpsum", bufs=2, space=bass.MemorySpace.PSUM) as ap,
                tc.tile_pool(name="epool", bufs=2) as ep,
                tc.tile_pool(name="wpool", bufs=2) as wp,
                tc.tile_pool(name="wtpool", bufs=2) as wtp,
                tc.tile_pool(name="scr", bufs=2) as scrp,
                tc.tile_pool(name="stat", bufs=2) as stp,
            ):
                for h in range(H):
                    qp, g = 64 * (h % 2), h // 2
                    E = ep.tile([128, 4, NT], F16, tag="E")
                    st = stp.tile([128, 44], F32, tag="st")
                    # st columns: 0:4 c_lo | 4:8 c_hi | 8:12 c_r | 12:16 t
                    # 16:20 u | 20:24 lam | 24:28 rcp_lam | 28:32 den
                    # 32:36 rcp_den | 36:40 tmp | 40:44 tmp2

                    # dots + exp (PE -> ACT), 4 i-tiles of 128 q rows
                    for it in range(4):
                        dps = dp.tile([128, NT], F32, tag="d")
                        for jh in range(2):
                            nc.tensor.matmul(
                                dps[:, 512 * jh:512 * (jh + 1)],
                                qT[qp:qp + 64, g, 128 * it:128 * (it + 1)],
                                kT[qp:qp + 64, g, 512 * jh:512 * (jh + 1)],
                                start=True, stop=True)
                        nc.scalar.activation(E[:, it, :], dps[:], AF.Exp, scale=SCALE)

                    # --- selection: bracketed counts + interpolation ---
                    # anchors on ACT: acc = sum(sign(E - u)) in [-1024, 1024];
                    # count c = (acc + 1024)/2, so in acc units the target
                    # rank is A_T = 2*KK - 1024 and slopes double (lam2).
                    for it in range(4):
                        scr = scrp.tile([128, NT], F16, tag="scr")
                        nc.scalar.activation(scr[:], E[:, it, :], AF.Sign,
                                             bias=blo[:], accum_out=st[:, it:it + 1])
                    for it in range(4):
                        scr = scrp.tile([128, NT], F16, tag="scr")
                        nc.scalar.activation(scr[:], E[:, it, :], AF.Sign,
                                             bias=bhi[:], accum_out=st[:, 4 + it:5 + it])
                    # lam2 = clip((acc_lo - acc_hi) * LAM0, 2*LAM_MIN, 2*LAM_MAX)
                    nc.vector.tensor_tensor(st[:, 36:40], st[:, 0:4], st[:, 4:8], AL.subtract)
                    nc.vector.tensor_scalar(out=st[:, 20:24], in0=st[:, 36:40],
                                            scalar1=LAM0, scalar2=2 * LAM_MIN,
                                            op0=AL.mult, op1=AL.max)
                    nc.vector.tensor_scalar(out=st[:, 20:24], in0=st[:, 20:24],
                                            scalar1=2 * LAM_MAX, scalar2=None, op0=AL.min)
                    nc.vector.reciprocal(st[:, 24:28], st[:, 20:24])
                    # t = (T0-W0) + (acc_lo - A_T) * rcp2
                    nc.vector.tensor_scalar(out=st[:, 36:40], in0=st[:, 0:4],
                                            scalar1=float(2 * KK - NT), scalar2=None,
                                            op0=AL.subtract)
                    nc.vector.tensor_tensor(st[:, 40:44], st[:, 36:40], st[:, 24:28], AL.mult)
                    nc.vector.tensor_scalar(out=st[:, 12:16], in0=st[:, 40:44],
                                            scalar1=T0 - W0, scalar2=None, op0=AL.add)
                    nc.scalar.activation(st[:, 16:20], st[:, 12:16], AF.Exp)

                    for _ in range(n_refine):
                        for it in range(4):
                            scr = scrp.tile([128, NT], F16, tag="scr")
                            nc.vector.tensor_scalar(
                                out=scr[:], in0=E[:, it, :],
                                scalar1=st[:, 16 + it:17 + it], scalar2=None,
                                op0=AL.is_ge, op1=AL.add,
                                accum_out=st[:, 8 + it:9 + it])
                        # t += (c - KK) * 2 * rcp2
                        nc.vector.tensor_scalar(out=st[:, 36:40], in0=st[:, 8:12],
                                                scalar1=float(KK), scalar2=2.0,
                                                op0=AL.subtract, op1=AL.mult)
                        nc.vector.tensor_tensor(st[:, 40:44], st[:, 36:40],
                                                st[:, 24:28], AL.mult)
                        nc.vector.tensor_tensor(st[:, 12:16], st[:, 12:16],
                                                st[:, 40:44], AL.add)
                        nc.scalar.activation(st[:, 16:20], st[:, 12:16], AF.Exp)

                    # --- mask + denominator, then normalize ---
                    W = wp.tile([128, 4, NT], F16, tag="W")
                    for it in range(4):
                        nc.vector.scalar_tensor_tensor(
                            out=W[:, it, :], in0=E[:, it, :],
                            scalar=st[:, 16 + it:17 + it], in1=E[:, it, :],
                            op0=AL.is_ge, op1=AL.mult,
                            accum_out=st[:, 28 + it:29 + it])
                    nc.vector.reciprocal(st[:, 32:36], st[:, 28:32])
                    for it in range(4):
                        nc.vector.tensor_scalar(
                            out=W[:, it, :], in0=W[:, it, :],
                            scalar1=st[:, 32 + it:33 + it], scalar2=None, op0=AL.mult)

                    # --- W^T via PE transpose, psum -> sbuf ---
                    WT = wtp.tile([128, 8, NQ], F16, tag="WT")
                    for jc in range(8):
                        tps = tp.tile([128, NQ], F16, tag="t")
                        for it in range(4):
                            nc.tensor.transpose(
                                tps[:, 128 * it:128 * (it + 1)],
                                W[:, it, 128 * jc:128 * (jc + 1)], ident[:])
                        if jc % 2 == 0:
                            nc.scalar.copy(WT[:, jc, :], tps[:])
                        else:
                            nc.vector.tensor_copy(WT[:, jc, :], tps[:])

                    # --- headout^T = sum_j V^T-chunks @ W^T ---
                    avp = ap.tile([64, NQ], F32, tag="av")
                    for jc in range(8):
                        nc.tensor.matmul(avp[:], v16[:, jc, 64 * h:64 * (h + 1)],
                                         WT[:, jc, :], start=(jc == 0), stop=(jc == 7))
                    nc.scalar.copy(hoT[qp:qp + 64, g, :], avp[:])

            # ---------- phase D: output projection + bias ----------
            with tc.tile_pool(name="ypsum", bufs=2, space=bass.MemorySpace.PSUM) as yp:
                for tt in range(4):
                    ps = yp.tile([128, DIM], F32, tag="y")
                    for g in range(4):
                        nc.tensor.matmul(ps[:], hoT[:, g, 128 * tt:128 * (tt + 1)],
                                         w16["wo"][:, g, :], start=(g == 0), stop=(g == 3))
                    nc.vector.tensor_tensor(ysb[:, tt, :], ps[:], bbc[:], AL.add)
                nc.sync.dma_start(y_d.rearrange("(t p) d -> p t d", p=128), ysb[:])

    nc.compile()
    return nc


_NC = None


def _get_nc():
    global _NC
    if _NC is None:
        _NC = build_nc()
    return _NC


def kernel(x, w_qkv, w_out, b_out):
    x = np.ascontiguousarray(np.asarray(x, dtype=np.float32))
    w_qkv = np.ascontiguousarray(np.asarray(w_qkv, dtype=np.float32))
    w_out = np.ascontiguousarray(np.asarray(w_out, dtype=np.float32))
    b_out = np.ascontiguousarray(
        np.broadcast_to(np.asarray(b_out, dtype=np.float32).reshape(1, DIM),
                        (128, DIM)))

    wq = np.ascontiguousarray(w_qkv[:, 0:DIM])
    wk = np.ascontiguousarray(w_qkv[:, DIM:2 * DIM])
    wv = np.ascontiguousarray(w_qkv[:, 2 * DIM:3 * DIM])

    in_maps = []
    for core in range(N_CORES):
        b, s = core // 2, core % 2
        in_maps.append({
            "x_full": x[b],
            "x_half": np.ascontiguousarray(x[b, NQ * s:NQ * (s + 1)]),
            "wq": wq, "wk": wk, "wv": wv, "wo": w_out, "bo": b_out,
        })

    res = run_bass_kernel_spmd(_get_nc(), in_maps, list(range(N_CORES)))

    y = np.empty((B, NT, DIM), dtype=np.float32)
    for core in range(N_CORES):
        b, s = core // 2, core % 2
        y[b, NQ * s:NQ * (s + 1)] = res.results[core]["y"]
    return y

